# revision 2
# baseline (speedup 1.0000x reference)
"""Trainium2 Bass kernel for the AF3-style diffusion loss — v3.

v2 -> v3: engine-overlap oriented schedule.
  - Phase split per rep: (1) all PE matmuls + ACT sqrts fill dx/dgt for all
    4 row tiles; (2) DVE/ACT downstream, issue-ordered so the ACT sigmoid
    block of tile t runs while the DVE threshold block of tile t-1 runs.
  - fp32r matmuls: 1 cycle/row (vs 4 for fp32) since free dim 512 >= 256.
  - Sigmoid outputs + adds in bf16 (DVE 2x mode for the e4 accumulation).
"""

import os
import numpy as np

B, A, T, APT = 2, 2048, 256, 8
NCORES = 8
RB = A // 4          # 512 rows per core
NT = RB // 128       # 4 row tiles per core
CH = 512             # matmul free-dim chunk (one PSUM bank)
OUTW = 20            # out cols: per tile t, t*5 + (c15, c30, ce15, ce30, bond)
SIGMA_DATA = 16.0
BUMP = 1e-3          # added to |xj|^2 so d^2 > 0 under matmul rounding
E0 = 0.25 * sum(1.0 / (1.0 + np.exp(-z)) for z in (0.5, 1.0, 2.0, 4.0))

_CACHE = {}
LAST_RESULTS = None  # test.py reads exec_time_ns from here
LAST_IN_MAPS = None


def _build_bass(reps=1):
    import concourse.bacc as bacc
    import concourse.mybir as mybir
    from concourse.tile import TileContext

    f32 = mybir.dt.float32
    f32r = mybir.dt.float32r
    bf16 = mybir.dt.bfloat16
    Alu = mybir.AluOpType
    AF = mybir.ActivationFunctionType

    nc = bacc.Bacc(None, target_bir_lowering=False)
    rows_d = nc.dram_tensor("rows", [5, 2 * NT * 128], f32, kind="ExternalInput")
    cols_d = nc.dram_tensor("cols", [5, 2 * A], f32, kind="ExternalInput")
    wb_d = nc.dram_tensor("wb", [128, NT * T], f32, kind="ExternalInput")
    out_d = nc.dram_tensor("out", [128, OUTW], f32, kind="ExternalOutput")

    with TileContext(nc) as tc:
        with (
            tc.tile_pool(name="const", bufs=1) as cp,
            tc.tile_pool(name="dpool", bufs=1) as dp,
            tc.tile_pool(name="work", bufs=2) as wp,
            tc.tile_pool(name="psum", bufs=2, space="PSUM") as pp,
        ):
            rows_sb = cp.tile([5, 2 * NT * 128], f32, name="rows_sb", tag="rows_sb")
            cols_sb = cp.tile([5, 2 * A], f32, name="cols_sb", tag="cols_sb")
            wb_sb = cp.tile([128, NT * T], f32, name="wb_sb", tag="wb_sb")
            outb = cp.tile([128, OUTW], f32, name="out_sb", tag="out_sb")
            nc.sync.dma_start(rows_sb[:], rows_d[:])
            nc.sync.dma_start(cols_sb[:], cols_d[:])
            nc.sync.dma_start(wb_sb[:], wb_d[:])
            rows_r = rows_sb[:]
            cols_r = cols_sb[:]

            def act_const(val, nm):
                st = cp.tile([128, 1], f32, name=nm + "_st", tag=nm + "_st")
                nc.vector.memset(st[:], val)
                fin = cp.tile([128, 1], f32, name=nm, tag=nm)
                nc.scalar.activation(fin[:], st[:], AF.Copy)
                return fin

            biaseps = act_const(1e-12, "biaseps")
            btau = [act_const(float(tau), f"btau{k}")
                    for k, tau in enumerate((0.5, 1.0, 2.0, 4.0))]

            for rep in range(reps):
                # -------- phase 1: d = sqrt(d^2) for all tiles, both coords
                dx = [None] * NT
                dgt = [None] * NT
                for t in range(NT):
                    dx[t] = dp.tile([128, A], f32, name=f"dx{t}_{rep}", tag=f"dx{t}")
                    dgt[t] = dp.tile([128, A], f32, name=f"dg{t}_{rep}", tag=f"dg{t}")
                for t in range(NT):
                    for s, dst in ((0, dx[t]), (1, dgt[t])):
                        lhsT = rows_r[:, (s * NT + t) * 128:(s * NT + t + 1) * 128]
                        for ch in range(A // CH):
                            ps = pp.tile([128, CH], f32,
                                         name=f"ps{s}_{rep}_{t}_{ch}", tag=f"ps{s}")
                            rhs = cols_r[:, s * A + ch * CH: s * A + (ch + 1) * CH]
                            nc.tensor.matmul(ps[:], lhsT, rhs, start=True, stop=True)
                            nc.scalar.activation(dst[:, ch * CH:(ch + 1) * CH], ps[:],
                                                 AF.Sqrt, bias=biaseps[:])

                # -------- phase 2a: delta / |delta| for all tiles (unblocks
                # the ACT sigmoid stream early)
                delta = [None] * NT
                adel = [None] * NT
                for t in range(NT):
                    delta[t] = wp.tile([128, A], f32, name=f"dl_{rep}_{t}", tag="delta")
                    nc.vector.tensor_sub(delta[t][:], dgt[t][:], dx[t][:])
                    adel[t] = wp.tile([128, A], f32, name=f"ad_{rep}_{t}", tag="adel")
                    nc.vector.scalar_tensor_tensor(
                        adel[t][:], delta[t][:], -1.0, delta[t][:],
                        op0=Alu.mult, op1=Alu.max)

                # ACT: all sigmoids, 4 per tile, bf16 outputs
                sg = {}
                for t in range(NT):
                    for k in range(4):
                        sg[t, k] = wp.tile([128, A], bf16, name=f"sg{k}_{rep}_{t}",
                                           tag=f"sg{k}")
                        nc.scalar.activation(sg[t, k][:], adel[t][:], AF.Sigmoid,
                                             bias=btau[k][:], scale=-1.0)

                # -------- phase 2b: per-tile DVE block
                for t in range(NT):
                    d2 = delta[t]  # square in place; delta dead afterwards
                    nc.vector.tensor_mul(d2[:], delta[t][:], delta[t][:])
                    blk = wp.tile([128, T], f32, name=f"bk_{rep}_{t}", tag="blk")
                    nc.vector.tensor_reduce(
                        blk[:], d2[:].rearrange("p (k e) -> p k e", e=APT),
                        axis=mybir.AxisListType.X, op=Alu.add)
                    scr2 = wp.tile([128, T], f32, name=f"s2_{rep}_{t}", tag="scr2")
                    nc.vector.scalar_tensor_tensor(
                        scr2[:], blk[:], 1.0, wb_sb[:, t * T:(t + 1) * T],
                        op0=Alu.mult, op1=Alu.mult,
                        accum_out=outb[:, t * 5 + 4:t * 5 + 5])
                    # e4 accumulation in bf16 (2x DVE), in-place tree into sg0
                    nc.vector.tensor_add(sg[t, 0][:], sg[t, 0][:], sg[t, 1][:])
                    nc.vector.tensor_add(sg[t, 2][:], sg[t, 2][:], sg[t, 3][:])
                    nc.vector.tensor_add(sg[t, 0][:], sg[t, 0][:], sg[t, 2][:])
                    e = sg[t, 0]
                    scr = wp.tile([128, A], f32, name=f"sc_{rep}_{t}", tag="scr")
                    nc.vector.tensor_scalar(
                        scr[:], dgt[t][:], 15.0, None, Alu.is_lt, Alu.add,
                        accum_out=outb[:, t * 5 + 0:t * 5 + 1])
                    nc.vector.tensor_scalar(
                        scr[:], dgt[t][:], 30.0, None, Alu.is_lt, Alu.add,
                        accum_out=outb[:, t * 5 + 1:t * 5 + 2])
                    nc.vector.scalar_tensor_tensor(
                        scr[:], dgt[t][:], 15.0, e[:], op0=Alu.is_lt, op1=Alu.mult,
                        accum_out=outb[:, t * 5 + 2:t * 5 + 3])
                    nc.vector.scalar_tensor_tensor(
                        scr[:], dgt[t][:], 30.0, e[:], op0=Alu.is_lt, op1=Alu.mult,
                        accum_out=outb[:, t * 5 + 3:t * 5 + 4])

            nc.sync.dma_start(out_d[:], outb[:])
    nc.compile()
    return nc


def _tok_features(isp, isd, isr, isl, tb, tm, npt):
    """Token->atom features, general in npt/tm. All numpy, O(A*T)."""
    cum = np.cumsum(npt, -1)
    start = cum - npt
    l = np.arange(A)
    ind = ((l[:, None] >= start[:, None, :]) & (l[:, None] < cum[:, None, :]))
    ind = ind.astype(np.float32)                      # [B,A,T] pure indicator
    oh = ind * tm[:, None, :]
    is_nuc = np.einsum('blt,bt->bl', oh, isd + isr)
    w_tok = 1.0 + isd * 5.0 + isr * 5.0 + isl * 10.0
    w_atom = np.einsum('blt,bt->bl', oh, w_tok)
    is_poly = isp + isd + isr
    tbm = tb * (is_poly[:, None, :] * isl[:, :, None]) * tm[:, None, :] * tm[:, :, None]
    wb_full = np.einsum('blt,btj->blj', ind, tbm)     # [B,A,T] bond row weights
    return oh, ind, is_nuc, w_atom, tbm, wb_full


def _mse_host(x, gt, gm, w_atom):
    """Weighted rigid align (Kabsch) of gt onto x + weighted MSE. Per sample."""
    denom = gm.sum()
    w_mean = (w_atom * gm).sum() / denom
    wm = (w_atom * gm)[:, None]
    mu = (gt * wm).sum(0) / denom / w_mean
    mu_gt = (x * wm).sum(0) / denom / w_mean
    xc = gt - mu
    xgc = x - mu_gt
    H = (xgc * wm).T @ xc
    U, _, Vh = np.linalg.svd(H)
    det = np.linalg.det(U @ Vh)
    s = np.array([1.0, 1.0, np.sign(det)])
    R = U @ (Vh * s[:, None])
    gt_al = xc @ R.T + mu_gt
    return (1.0 / 3.0) * (((x - gt_al) ** 2).sum(-1) * w_atom * gm).sum() / denom


def _numpy_fallback(x, gt, gm, isp, isd, isr, isl, tb, tm, npt, t):
    """Full-precision numpy port of the reference; used only when the inputs
    fall outside the fast-path assumptions (non-uniform atoms/masks)."""
    oh, ind, is_nuc, w_atom, tbm, wb_full = _tok_features(isp, isd, isr, isl, tb, tm, npt)
    sig = lambda z: 1.0 / (1.0 + np.exp(-z))
    loss = 0.0
    for b in range(B):
        d = x[b][:, None, :] - x[b][None, :, :]
        dx = np.sqrt((d * d).sum(-1) + 1e-12)
        d = gt[b][:, None, :] - gt[b][None, :, :]
        dg = np.sqrt((d * d).sum(-1) + 1e-12)
        pm = gm[b][:, None] * gm[b][None, :]
        bm = ind[b] @ tbm[b] @ ind[b].T
        m = bm * pm
        lb = (((dx - dg) ** 2) * m).sum() / m.sum()
        dd = np.abs(dg - dx)
        e = 0.25 * (sig(0.5 - dd) + sig(1.0 - dd) + sig(2.0 - dd) + sig(4.0 - dd))
        c = (dg < 30) * is_nuc[b][:, None] + (dg < 15) * (1.0 - is_nuc[b][:, None])
        m2 = (1.0 - np.eye(A)) * pm
        msum = m2.sum()
        ll = 1.0 - ((c * e * m2).sum() / msum) / ((c * m2).sum() / msum)
        lm = _mse_host(x[b], gt[b], gm[b], w_atom[b])
        wt = (t[b] ** 2 + SIGMA_DATA ** 2) / (t[b] + SIGMA_DATA) ** 2
        loss += wt * (lm + lb) + ll
    return np.float32(loss / B)


def _make_in_maps(x, gt, wb_full):
    in_maps = []
    for c in range(NCORES):
        b, r = divmod(c, NT)
        rows = np.empty((5, 2 * NT * 128), np.float32)
        cols = np.empty((5, 2 * A), np.float32)
        for s, coords in ((0, x[b]), (1, gt[b])):
            nrm = (coords * coords).sum(-1)
            blkc = coords[RB * r:RB * (r + 1)]          # [512, 3]
            nb = nrm[RB * r:RB * (r + 1)]
            sl = slice(s * NT * 128, (s + 1) * NT * 128)
            rows[0:3, sl] = blkc.T
            rows[3, sl] = nb
            rows[4, sl] = 1.0
            cl = slice(s * A, (s + 1) * A)
            cols[0:3, cl] = -2.0 * coords.T
            cols[3, cl] = 1.0
            cols[4, cl] = nrm + BUMP
        wb = np.empty((128, NT * T), np.float32)
        for t in range(NT):
            wb[:, t * T:(t + 1) * T] = wb_full[b][RB * r + 128 * t:
                                                  RB * r + 128 * (t + 1)]
        in_maps.append({"rows": rows, "cols": cols, "wb": wb})
    return in_maps


def kernel(x, gt_atom_positions, gt_atom_mask, is_protein, is_dna, is_rna,
           is_ligand, token_bonds, token_mask, num_atoms_per_token, t):
    global LAST_RESULTS, LAST_IN_MAPS
    f = np.asarray
    x = f(x, np.float32)
    gt = f(gt_atom_positions, np.float32)
    gm = f(gt_atom_mask, np.float32)
    isp, isd, isr, isl = (f(v, np.float32) for v in
                          (is_protein, is_dna, is_rna, is_ligand))
    tb = f(token_bonds, np.float32)
    tm = f(token_mask, np.float32)
    npt = f(num_atoms_per_token, np.int32)
    t = f(t, np.float32)

    fast = bool(np.all(npt == APT)) and bool(np.all(gm == 1.0))
    if not fast:
        return _numpy_fallback(x, gt, gm, isp, isd, isr, isl, tb, tm, npt, t)

    oh, ind, is_nuc, w_atom, tbm, wb_full = _tok_features(isp, isd, isr, isl, tb, tm, npt)
    in_maps = _make_in_maps(x, gt, wb_full)

    if "nc" not in _CACHE:
        _CACHE["nc"] = _build_bass()
    os.environ.setdefault("BASS_NEVER_TRACE", "1")
    from concourse.bass_utils import run_bass_kernel_spmd
    res = run_bass_kernel_spmd(_CACHE["nc"], in_maps, core_ids=list(range(NCORES)))
    LAST_RESULTS = res
    LAST_IN_MAPS = in_maps

    # Host combine. Device layout: out[:, t*5 + (c15, c30, ce15, ce30, bond)]
    # for row-tile t; row base = 512*r + 128*t within sample b = core//4.
    loss = 0.0
    for b in range(B):
        s15 = np.empty(A, np.float64); s30 = np.empty(A, np.float64)
        s15e = np.empty(A, np.float64); s30e = np.empty(A, np.float64)
        bond_total = 0.0
        for r in range(NT):
            o = res.results[NT * b + r]["out"]  # [128, OUTW]
            for t_ in range(NT):
                base = RB * r + 128 * t_
                s15[base:base + 128] = o[:, t_ * 5 + 0]
                s30[base:base + 128] = o[:, t_ * 5 + 1]
                s15e[base:base + 128] = o[:, t_ * 5 + 2]
                s30e[base:base + 128] = o[:, t_ * 5 + 3]
                bond_total += float(o[:, t_ * 5 + 4].astype(np.float64).sum())
        nuc = is_nuc[b].astype(np.float64)
        c_rows = s15 + nuc * (s30 - s15) - 1.0
        ce_rows = 0.25 * (s15e + nuc * (s30e - s15e)) - E0
        ll = 1.0 - ce_rows.sum() / c_rows.sum()
        a_i = ind[b].T @ gm[b].astype(np.float32)     # atoms per token (masked)
        bond_den = float(a_i @ tbm[b] @ a_i)
        lb = bond_total / bond_den
        lm = _mse_host(x[b], gt[b], gm[b], w_atom[b])
        wt = (t[b] ** 2 + SIGMA_DATA ** 2) / (t[b] + SIGMA_DATA) ** 2
        loss += wt * (lm + lb) + ll
    return np.float32(loss / B)
